# revision 5
# baseline (speedup 1.0000x reference)
"""Trainium2 Bass kernel for location-sensitive (Bahdanau) attention, one decoder step.

Reference computation (per example b):
    conv_feat = conv1d(att_w[b], conv_k, pad=50)            # [T, C]
    hidden    = tanh(enc[b] @ W1e + dec[b] @ W1d + W_b + conv_feat @ Wc)   # [T, A]
    att       = softmax(2 * hidden @ V)                      # [T]
    ctx       = att @ enc[b]                                 # [D]

Strategy: data-parallel over batch (64 examples -> 8 cores x 8 examples).
All heavy math in bf16 on the TensorEngine with f32 PSUM accumulation.

Device formulation (per example):
  pass1: hiddenT[A, T] accumulated in PSUM from
           -  G_ext[102, A].T @ AWS[102, T]   (conv via im2col of att_w, with
              a ones-row so dec@W1d + W_b folds in as row 101)
           -  W1e[128,A].T @ encT[128, T]  x 8 K-chunks
         tanh on ScalarE -> hiddenT in SBUF (bf16)
         energy[1, T] = V.T @ hiddenT   (PE, 4 K-chunks)
  softmax: per-example on [1, T] row: max, exp(2x - 2max) w/ fused accum-sum,
         reciprocal, scale. (DVE/ACT)
  pass2: attT columns via PE transpose; ctx[1, D] = attT.T @ encN chunks (PE).
"""

import numpy as np
import ml_dtypes

BF16 = ml_dtypes.bfloat16

B, T, D, A, C, KW = 64, 1500, 1024, 512, 10, 101
PAD = (KW - 1) // 2
NCORES = 8
BPC = B // NCORES  # examples per core
P = 128
NK = D // P   # 8 K-chunks over D
NA = A // P   # 4 A-tiles
TCH = [(0, 512), (512, 512), (1024, T - 1024)]          # pass1 T chunks
T128 = [(i * P, min(P, T - i * P)) for i in range((T + P - 1) // P)]  # pass2 chunks
N12 = len(T128)

_CACHE = {}


def _build_nc():
    import concourse.bass as bass
    import concourse.mybir as mybir
    import concourse.tile as tile
    from concourse import bacc

    dt = mybir.dt
    AF = mybir.ActivationFunctionType

    nc = bacc.Bacc(
        "TRN2",
        target_bir_lowering=False,
        debug=False,
        enable_asserts=False,
        num_devices=NCORES,
    )

    # ---- DRAM I/O (per-core shard shapes) ----
    encT = nc.dram_tensor("encT", [BPC, D, T], dt.bfloat16, kind="ExternalInput").ap()
    encN = nc.dram_tensor("encN", [BPC, T, D], dt.bfloat16, kind="ExternalInput").ap()
    aws = nc.dram_tensor("aws", [BPC, KW + 1, T], dt.bfloat16, kind="ExternalInput").ap()
    w1e = nc.dram_tensor("w1e", [P, NK, A], dt.bfloat16, kind="ExternalInput").ap()
    w1d = nc.dram_tensor("w1d", [P, NK, A], dt.bfloat16, kind="ExternalInput").ap()
    decT = nc.dram_tensor("decT", [P, NK, BPC], dt.bfloat16, kind="ExternalInput").ap()
    wb = nc.dram_tensor("wb", [1, A], dt.bfloat16, kind="ExternalInput").ap()
    vw = nc.dram_tensor("vw", [P, NA], dt.bfloat16, kind="ExternalInput").ap()
    ckm = nc.dram_tensor("ckm", [C, KW], dt.bfloat16, kind="ExternalInput").ap()
    wcm = nc.dram_tensor("wcm", [C, A], dt.bfloat16, kind="ExternalInput").ap()
    ones8 = nc.dram_tensor("ones8", [1, BPC], dt.bfloat16, kind="ExternalInput").ap()
    out_att = nc.dram_tensor("out_att", [BPC, T], dt.float32, kind="ExternalOutput").ap()
    out_ctx = nc.dram_tensor("out_ctx", [BPC, D], dt.float32, kind="ExternalOutput").ap()

    with tile.TileContext(nc) as tc:
        with (
            tc.tile_pool(name="const", bufs=1) as cpool,
            tc.tile_pool(name="gext", bufs=1) as gpool,
            tc.tile_pool(name="work", bufs=1) as wpool,
            tc.tile_pool(name="psum", bufs=1, space="PSUM") as ppool,
        ):
            # ---- load constants ----
            w1sb = cpool.tile([P, NK, A], dt.bfloat16, name="w1sb")
            nc.sync.dma_start(w1sb[:], w1e[:])
            vsb = cpool.tile([P, NA], dt.bfloat16, name="vsb")
            nc.sync.dma_start(vsb[:], vw[:])
            ident = cpool.tile([1, 1], dt.bfloat16, name="ident")
            nc.vector.memset(ident[:], 1.0)

            # ---- setup: G = ck.T @ Wc  [KW, A]; dec_row = dec @ W1d + W_b [BPC, A]
            cksb = cpool.tile([C, KW], dt.bfloat16, name="cksb")
            nc.sync.dma_start(cksb[:], ckm[:])
            wcsb = cpool.tile([C, A], dt.bfloat16, name="wcsb")
            nc.sync.dma_start(wcsb[:], wcm[:])
            w1dsb = cpool.tile([P, NK, A], dt.bfloat16, name="w1dsb")
            nc.sync.dma_start(w1dsb[:], w1d[:])
            decTsb = cpool.tile([P, NK, BPC], dt.bfloat16, name="decTsb")
            nc.sync.dma_start(decTsb[:], decT[:])
            ones8sb = cpool.tile([1, BPC], dt.bfloat16, name="ones8sb")
            nc.sync.dma_start(ones8sb[:], ones8[:])
            wbsb = cpool.tile([1, A], dt.bfloat16, name="wbsb")
            nc.sync.dma_start(wbsb[:], wb[:])

            g_ps = ppool.tile([KW, A], dt.float32, name="g_ps", tag="hid0")
            nc.tensor.matmul(g_ps[:], cksb[:], wcsb[:], start=True, stop=True)
            g_sb = cpool.tile([KW, A], dt.bfloat16, name="g_sb")
            nc.scalar.copy(g_sb[:], g_ps[:])

            dec_ps = ppool.tile([BPC, A], dt.float32, name="dec_ps", tag="hid1")
            nc.tensor.matmul(
                dec_ps[:], ones8sb[:], wbsb[:], start=True, stop=False
            )
            for kc in range(NK):
                nc.tensor.matmul(
                    dec_ps[:],
                    decTsb[:, kc, :],
                    w1dsb[:, kc, :],
                    start=False,
                    stop=(kc == NK - 1),
                )
            dec_sb = cpool.tile([BPC, A], dt.bfloat16, name="dec_sb")
            nc.scalar.copy(dec_sb[:], dec_ps[:])

            # G_ext per example: rows 0..100 = G, row 101 = dec_row_b (incl W_b)
            gext = []
            for b in range(BPC):
                gt = gpool.tile([KW + 1, A], dt.bfloat16, name=f"gext{b}")
                nc.sync.dma_start(gt[0:KW, :], g_sb[:])
                nc.sync.dma_start(gt[KW : KW + 1, :], dec_sb[b : b + 1, :])
                gext.append(gt)

            # ---- per-example state emitted across the main loop ----
            att_bf = [None] * BPC
            attT = [None] * BPC

            def emit_pass1(b):
                awsb = wpool.tile([KW + 1, T], dt.bfloat16, name=f"aws{b}", tag="awsb", bufs=2)
                nc.sync.dma_start(awsb[:], aws[b])
                e_sb = wpool.tile([1, T], dt.float32, name=f"esb{b}", tag="e_sb", bufs=2)
                encT_r = encT[b].rearrange("(c p) t -> p c t", p=P)
                for t0, tn in TCH:
                    slab = wpool.tile([P, NK, 512], dt.bfloat16, name=f"sl{b}_{t0}", tag="slab", bufs=3)
                    nc.sync.dma_start(slab[:, :, :tn], encT_r[:, :, t0 : t0 + tn])
                    hts = []
                    for at in range(NA):
                        hid = ppool.tile(
                            [P, 512], dt.float32, name=f"h{b}_{t0}_{at}", tag=f"hid{at}"
                        )
                        nc.tensor.matmul(
                            hid[:, :tn],
                            gext[b][:, at * P : (at + 1) * P],
                            awsb[:, t0 : t0 + tn],
                            start=True,
                            stop=False,
                        )
                        for kc in range(NK):
                            nc.tensor.matmul(
                                hid[:, :tn],
                                w1sb[:, kc, at * P : (at + 1) * P],
                                slab[:, kc, :tn],
                                start=False,
                                stop=(kc == NK - 1),
                            )
                        ht = wpool.tile(
                            [P, 512], dt.bfloat16, name=f"ht{b}_{t0}_{at}",
                            tag=f"hidsb{at}", bufs=2,
                        )
                        nc.scalar.activation(ht[:, :tn], hid[:, :tn], AF.Tanh)
                        hts.append(ht)
                    eps = ppool.tile([1, 512], dt.float32, name=f"e{b}_{t0}", tag="eps")
                    for at in range(NA):
                        nc.tensor.matmul(
                            eps[0:1, :tn],
                            vsb[:, at : at + 1],
                            hts[at][:, :tn],
                            start=(at == 0),
                            stop=(at == NA - 1),
                        )
                    nc.scalar.copy(e_sb[0:1, t0 : t0 + tn], eps[0:1, :tn])
                return e_sb

            def emit_softmax(b, e_sb):
                m = wpool.tile([1, 1], dt.float32, name=f"m{b}", tag="m", bufs=2)
                nc.vector.reduce_max(m[:], e_sb[:], axis=mybir.AxisListType.X)
                negm2 = wpool.tile([1, 1], dt.float32, name=f"nm{b}", tag="negm2", bufs=2)
                nc.vector.tensor_scalar_mul(negm2[:], m[:], -2.0)
                unorm = wpool.tile([1, T], dt.float32, name=f"un{b}", tag="unorm", bufs=2)
                s = wpool.tile([1, 1], dt.float32, name=f"s{b}", tag="s", bufs=2)
                nc.scalar.activation(
                    unorm[:], e_sb[:], AF.Exp,
                    bias=negm2[:], scale=2.0, accum_out=s[:],
                )
                r = wpool.tile([1, 1], dt.float32, name=f"r{b}", tag="r", bufs=2)
                nc.vector.reciprocal(r[:], s[:])
                att_f = wpool.tile([1, T], dt.float32, name=f"af{b}", tag="att_f", bufs=2)
                nc.vector.tensor_scalar_mul(att_f[:], unorm[:], r[:])
                nc.sync.dma_start(out_att[b : b + 1, :], att_f[:])
                abf = wpool.tile([1, T], dt.bfloat16, name=f"ab{b}", tag="att_bf", bufs=3)
                nc.vector.tensor_scalar_mul(abf[:], unorm[:], r[:])
                att_bf[b] = abf

            def emit_transposes(b):
                # bf16 PSUM writes must be 4B-aligned: pad each column to a 4B slot
                trp = ppool.tile([P, N12, 2], dt.bfloat16, name=f"tr{b}", tag="trp")
                for i, (t0, tn) in enumerate(T128):
                    nc.tensor.transpose(
                        trp[0:tn, i, 0:1], att_bf[b][0:1, t0 : t0 + tn], ident[:]
                    )
                at = wpool.tile([P, N12], dt.bfloat16, name=f"aT{b}", tag="attT", bufs=2)
                nc.scalar.copy(at[:, 0 : N12 - 1], trp[:, 0 : N12 - 1, 0])
                ltn = T128[-1][1]
                nc.scalar.copy(at[0:ltn, N12 - 1 : N12], trp[0:ltn, N12 - 1, 0:1])
                attT[b] = at

            def emit_pass2(b):
                emit_transposes(b)
                ctx_ps = ppool.tile([1, D], dt.float32, name=f"cx{b}", tag="ctx")
                for i, (t0, tn) in enumerate(T128):
                    slab = wpool.tile([P, D], dt.bfloat16, name=f"s2{b}_{i}", tag="encn", bufs=16)
                    nc.sync.dma_start(slab[:tn, :], encN[b, t0 : t0 + tn, :])
                    for ns in range(2):
                        nc.tensor.matmul(
                            ctx_ps[0:1, ns * 512 : (ns + 1) * 512],
                            attT[b][0:tn, i : i + 1],
                            slab[0:tn, ns * 512 : (ns + 1) * 512],
                            start=(i == 0),
                            stop=(i == N12 - 1),
                        )
                ctx_sb = wpool.tile([1, D], dt.float32, name=f"cs{b}", tag="ctx_sb", bufs=2)
                nc.vector.tensor_copy(ctx_sb[:], ctx_ps[:])
                nc.sync.dma_start(out_ctx[b : b + 1, :], ctx_sb[:])

            # main interleave: pass2(b-1) emitted after pass1(b) so PE never
            # waits on softmax, and encN DMA spreads across the whole kernel
            for b in range(BPC):
                e_sb = emit_pass1(b)
                emit_softmax(b, e_sb)
                if b >= 1:
                    emit_pass2(b - 1)
            emit_pass2(BPC - 1)

    nc.compile()
    return nc


def _prep_in_maps(inputs):
    enc = np.asarray(inputs["enc_out"], dtype=np.float32)          # [B, T, D]
    dec = np.asarray(inputs["dec_out"], dtype=np.float32)[:, 0, :]  # [B, D]
    aw = np.asarray(inputs["att_weights_step"], dtype=np.float32)  # [B, T]
    Ww = np.asarray(inputs["W_w"], dtype=np.float32)               # [2D, A]
    Wb = np.asarray(inputs["W_b"], dtype=np.float32)               # [A]
    Wc = np.asarray(inputs["Wc_w"], dtype=np.float32)              # [C, A]
    Vw = np.asarray(inputs["V_w"], dtype=np.float32)               # [A, 1]
    ck = np.asarray(inputs["conv_k"], dtype=np.float32)            # [C, 1, KW]

    encN = np.ascontiguousarray(enc).astype(BF16)                     # [B, T, D]
    encT = np.ascontiguousarray(enc.transpose(0, 2, 1)).astype(BF16)  # [B, D, T]

    awp = np.pad(aw, ((0, 0), (PAD, PAD)))
    sw = np.lib.stride_tricks.sliding_window_view(awp, T, axis=1)  # [B, KW, T]
    aws = np.empty((B, KW + 1, T), dtype=BF16)
    aws[:, :KW, :] = sw.astype(BF16)
    aws[:, KW, :] = np.ones((B, T), dtype=BF16)

    w1e = np.ascontiguousarray(
        Ww[:D].reshape(NK, P, A).transpose(1, 0, 2)
    ).astype(BF16)  # [P, NK, A]
    w1d = np.ascontiguousarray(
        Ww[D:].reshape(NK, P, A).transpose(1, 0, 2)
    ).astype(BF16)
    vw = np.ascontiguousarray(Vw[:, 0].reshape(NA, P).T).astype(BF16)  # [P, NA]
    ckm = np.ascontiguousarray(ck[:, 0, :]).astype(BF16)  # [C, KW]
    wcm = np.ascontiguousarray(Wc).astype(BF16)
    wbm = np.ascontiguousarray(Wb.reshape(1, A)).astype(BF16)
    ones8 = np.ones((1, BPC), dtype=BF16)

    in_maps = []
    for c in range(NCORES):
        sl = slice(c * BPC, (c + 1) * BPC)
        dec_c = dec[sl]  # [BPC, D]
        decT = np.ascontiguousarray(
            dec_c.T.reshape(NK, P, BPC).transpose(1, 0, 2)
        ).astype(BF16)  # [P, NK, BPC]
        in_maps.append(
            {
                "encT": np.ascontiguousarray(encT[sl]),
                "encN": np.ascontiguousarray(encN[sl]),
                "aws": np.ascontiguousarray(aws[sl]),
                "w1e": w1e,
                "w1d": w1d,
                "decT": decT,
                "wb": wbm,
                "vw": vw,
                "ckm": ckm,
                "wcm": wcm,
                "ones8": ones8,
            }
        )
    return in_maps


def get_nc():
    if "nc" not in _CACHE:
        _CACHE["nc"] = _build_nc()
    return _CACHE["nc"]


def kernel(**inputs):
    from concourse.bass_utils import run_bass_kernel_spmd

    nc = get_nc()
    in_maps = _prep_in_maps(inputs)
    res = run_bass_kernel_spmd(nc, in_maps, core_ids=list(range(NCORES)))
    att = np.concatenate([res.results[c]["out_att"] for c in range(NCORES)], axis=0)
    ctx = np.concatenate([res.results[c]["out_ctx"] for c in range(NCORES)], axis=0)
    return ctx[:, None, :].astype(np.float32), att.astype(np.float32)
